# revision 18
# baseline (speedup 1.0000x reference)
"""Trainium2 Bass kernel for DynamicTokenMixing (16-head attention, N=4096, C=1024).

Sharding: head-parallel across 8 NeuronCores, 2 heads per core. Each core
computes q/k/v projections for its 2 heads, full attention for those heads,
and a partial output projection (row-parallel over Wproj); the host sums the
8 partials and adds the bias.

Per-core dataflow (all layouts chosen so no on-chip transposes of the
attention matrix are needed):
  qT, kT   = (x @ Wq_cols).T, (x @ Wkv_kcols).T     [dual-head stacked, 128 x 4096]
  vT       = (x @ Wkv_vcols).T  -> PE-transposed to v tiles [j, d] (+ ones col)
  ST[j,i]  = sum_d k[j,d] q[i,d]          (scores, transposed layout)
  ET       = exp(0.5 * ST)                (0.5 = gpd_ratio^2 * d^-0.5)
  AV^T     = sum_j v_ext[j,:]^T ET[j,:]   (row 64 = softmax denominator l[i])
  outT     = AV^T[0:64] * (1/l) (broadcast)
  out      = sum_h outT_h.T @ Wproj_rows_h   (partial; host adds across cores)
"""

import numpy as np

import concourse.bass as bass
import concourse.mybir as mybir
import concourse.tile as tile
from concourse import bacc
from concourse.bass_utils import run_bass_kernel_spmd
from concourse.masks import make_identity

F32 = mybir.dt.float32
F32R = mybir.dt.float32r

N = 4096          # tokens
C = 1024          # model dim
D = 64            # head dim
NHEADS = 16
GPD = 2
NCORES = 8
NJ = N // 128     # 32 key tiles
NCT = C // 128    # 8 contraction tiles
STRIP = 512       # query-strip width
NSTRIP = N // STRIP
JB = 2            # key tiles batched per exp instruction
SCORE_SCALE = GPD * GPD * (D ** -0.5)  # 0.5


def build_nc():
    nc = bacc.Bacc("TRN2", target_bir_lowering=False, debug=False,
                   num_devices=NCORES)
    xT = nc.declare_dram_parameter("xT", [C, N], F32R, isOutput=False)
    wq = nc.declare_dram_parameter("wq", [C, 128], F32R, isOutput=False)
    wk = nc.declare_dram_parameter("wk", [C, 128], F32R, isOutput=False)
    wv = nc.declare_dram_parameter("wv", [C, 128], F32R, isOutput=False)
    wpa = nc.declare_dram_parameter("wpa", [D, C], F32R, isOutput=False)
    wpb = nc.declare_dram_parameter("wpb", [D, C], F32R, isOutput=False)
    out = nc.declare_dram_parameter("out", [N, C], F32, isOutput=True)

    xT_r = xT[:].rearrange("(t p) n -> p t n", p=128)    # [128, 8, 4096]
    out_r = out[:].rearrange("(t p) o -> t p o", p=128)  # [32, 128, 1024]

    with tile.TileContext(nc) as tc:
        with (
            nc.allow_low_precision(reason="fp32r (tf32) matmul inputs by design"),
            tc.tile_pool(name="persist", bufs=1) as persist,
            tc.tile_pool(name="small", bufs=4) as small,
        ):
            wq_sb = persist.tile([128, NCT, 128], F32R)
            wk_sb = persist.tile([128, NCT, 128], F32R)
            wv_sb = persist.tile([128, NCT, 128], F32R)
            wpa_sb = persist.tile([D, C], F32R)
            wpb_sb = persist.tile([D, C], F32R)
            qT = persist.tile([128, N], F32R)      # rows 0-63 head A, 64-127 head B
            kT = persist.tile([128, N], F32R)
            v_sb = persist.tile([128, NJ, 130], F32R)  # [j, (vA|1|vB|1)]
            outT_A = persist.tile([D, N], F32R)
            outT_B = persist.tile([D, N], F32R)
            ident = persist.tile([128, 128], F32)
            ones_f = persist.tile([128, D], F32)
            nc.gpsimd.memset(ones_f[:], 1.0)
            ones_t = persist.tile([65, D], F32R)
            nc.vector.tensor_copy(ones_t[:], ones_f[0:65, :])

            nc.sync.dma_start(wq_sb[:], wq[:].rearrange("(t p) m -> p t m", p=128))
            nc.sync.dma_start(wk_sb[:], wk[:].rearrange("(t p) m -> p t m", p=128))
            nc.sync.dma_start(wv_sb[:], wv[:].rearrange("(t p) m -> p t m", p=128))
            nc.sync.dma_start(wpa_sb[:], wpa[:])
            nc.sync.dma_start(wpb_sb[:], wpb[:])
            make_identity(nc, ident[:])
            nc.vector.tensor_copy(
                v_sb[:, :, 64], ones_f[0:128, 0:1].to_broadcast((128, NJ)))
            nc.vector.tensor_copy(
                v_sb[:, :, 129], ones_f[0:128, 0:1].to_broadcast((128, NJ)))

            # ---- Phase 1: qT/kT/vT projections; v_sb natural-layout tiles ----
            with (
                tc.tile_pool(name="ph1_sb", bufs=2) as ph1_sb,
                tc.tile_pool(name="ph1_big", bufs=1) as ph1_big,
                tc.tile_pool(name="ph1_ps", bufs=2, space="PSUM") as ph1_ps,
            ):
                vT = ph1_big.tile([128, N], F32)
                for i in range(NSTRIP):
                    sl = bass.ts(i, STRIP)
                    xt = ph1_sb.tile([128, NCT, STRIP], F32R, tag="xt")
                    nc.sync.dma_start(xt[:], xT_r[:, :, sl])
                    q_ps = ph1_ps.tile([128, STRIP], F32, tag="q")
                    k_ps = ph1_ps.tile([128, STRIP], F32, tag="k")
                    v_ps = ph1_ps.tile([128, STRIP], F32, tag="v")
                    for c in range(NCT):
                        st, sp = (c == 0), (c == NCT - 1)
                        nc.tensor.matmul(q_ps[:], wq_sb[:, c, :], xt[:, c, :],
                                         start=st, stop=sp)
                        nc.tensor.matmul(k_ps[:], wk_sb[:, c, :], xt[:, c, :],
                                         start=st, stop=sp)
                        nc.tensor.matmul(v_ps[:], wv_sb[:, c, :], xt[:, c, :],
                                         start=st, stop=sp)
                    nc.vector.tensor_copy(qT[:, sl], q_ps[:])
                    nc.vector.tensor_copy(kT[:, sl], k_ps[:])
                    nc.vector.tensor_copy(vT[:, sl], v_ps[:])
                with tc.tile_pool(name="tp_ps", bufs=2, space="PSUM") as tp_ps:
                    for j in range(NJ):
                        tp = tp_ps.tile([128, 128], F32, tag="tp")
                        nc.tensor.transpose(tp[:], vT[:, bass.ts(j, 128)], ident[:])
                        nc.vector.tensor_copy(v_sb[:, j, 0:64], tp[:, 0:64])
                        nc.vector.tensor_copy(v_sb[:, j, 65:129], tp[:, 64:128])

            # ---- Phase 2: attention (both heads interleaved per strip) ----
            with (
                tc.tile_pool(name="att_et", bufs=3) as et_pool,
                tc.tile_pool(name="att_st", bufs=1, space="PSUM") as st_pool,
                tc.tile_pool(name="att_av", bufs=1, space="PSUM") as av_pool,
                tc.tile_pool(name="att_bc", bufs=2, space="PSUM") as bc_pool,
            ):
                heads = ((0, slice(0, 64)), (1, slice(64, 128)))
                for i in range(NSTRIP):
                    sl = bass.ts(i, STRIP)
                    av = {h: av_pool.tile([65, STRIP], F32, tag=f"av{h}",
                                          name=f"av{h}")
                          for h, _ in heads}
                    for jp in range(NJ // JB):
                        for h, hs in heads:
                            st = st_pool.tile([128, JB * STRIP], F32, tag=f"st{h}")
                            for u in range(JB):
                                j = JB * jp + u
                                nc.tensor.matmul(
                                    st[:, bass.ts(u, STRIP)],
                                    kT[hs, bass.ts(j, 128)],
                                    qT[hs, sl],
                                    start=True, stop=True,
                                )
                            et = et_pool.tile([128, JB * STRIP], F32R, tag=f"et{h}")
                            nc.scalar.activation(
                                et[:], st[:],
                                mybir.ActivationFunctionType.Exp,
                                scale=SCORE_SCALE,
                            )
                            for u in range(JB):
                                j = JB * jp + u
                                nc.tensor.matmul(
                                    av[h][:],
                                    v_sb[:, j, h * 65:h * 65 + 65],
                                    et[:, bass.ts(u, STRIP)],
                                    start=(j == 0), stop=(j == NJ - 1),
                                    skip_group_check=True,
                                )
                    for h, outT_h in ((0, outT_A), (1, outT_B)):
                        stage = small.tile([65, STRIP], F32, tag="stage")
                        nc.vector.tensor_copy(stage[:], av[h][:])
                        rec_r = small.tile([65, STRIP], F32R, tag="rec_r")
                        nc.vector.reciprocal(rec_r[64:65, :], stage[64:65, :])
                        bc = bc_pool.tile([64, STRIP], F32, tag="bc")
                        nc.tensor.matmul(bc[:], ones_t[64:65, :],
                                         rec_r[64:65, :], start=True, stop=True)
                        nc.vector.tensor_mul(outT_h[:, sl], stage[0:64, :], bc[:])

            # ---- Phase 3: output projection (partial over this core's heads) ----
            with (
                tc.tile_pool(name="pr_sb", bufs=3) as pr_sb,
                tc.tile_pool(name="pr_ps", bufs=2, space="PSUM") as pr_ps,
            ):
                for it in range(N // 128):
                    pp = pr_ps.tile([128, C], F32, tag="pp")
                    for oc in range(C // STRIP):
                        osl = bass.ts(oc, STRIP)
                        nc.tensor.matmul(pp[:, osl], outT_A[:, bass.ts(it, 128)],
                                         wpa_sb[:, osl], start=True, stop=False)
                        nc.tensor.matmul(pp[:, osl], outT_B[:, bass.ts(it, 128)],
                                         wpb_sb[:, osl], start=False, stop=True)
                    ob = pr_sb.tile([128, C], F32, tag="ob")
                    nc.vector.tensor_copy(ob[:], pp[:])
                    nc.sync.dma_start(out_r[it], ob[:])
    nc.finalize()
    return nc


def _colk(h):
    base = h * D if h < 8 else 2 * 512 + (h - 8) * D
    return slice(base, base + D)


def _colv(h):
    base = 512 + h * D if h < 8 else 3 * 512 + (h - 8) * D
    return slice(base, base + D)


def make_in_maps(x, Wq, Wkv, Wproj):
    x = np.asarray(x, np.float32).reshape(N, C)
    Wq = np.asarray(Wq, np.float32)
    Wkv = np.asarray(Wkv, np.float32)
    Wproj = np.asarray(Wproj, np.float32)
    xT = np.ascontiguousarray(x.T)
    in_maps = []
    for core in range(NCORES):
        h0, h1 = 2 * core, 2 * core + 1
        in_maps.append({
            "xT": xT,
            "wq": np.ascontiguousarray(
                np.concatenate([Wq[:, h0 * D:(h0 + 1) * D],
                                Wq[:, h1 * D:(h1 + 1) * D]], axis=1)),
            "wk": np.ascontiguousarray(
                np.concatenate([Wkv[:, _colk(h0)], Wkv[:, _colk(h1)]], axis=1)),
            "wv": np.ascontiguousarray(
                np.concatenate([Wkv[:, _colv(h0)], Wkv[:, _colv(h1)]], axis=1)),
            "wpa": np.ascontiguousarray(Wproj[h0 * D:(h0 + 1) * D, :]),
            "wpb": np.ascontiguousarray(Wproj[h1 * D:(h1 + 1) * D, :]),
        })
    return in_maps


_NC = None


def _get_nc():
    global _NC
    if _NC is None:
        _NC = build_nc()
    return _NC


def run_spmd(in_maps, **kwargs):
    return run_bass_kernel_spmd(_get_nc(), in_maps, list(range(NCORES)), **kwargs)


def kernel(x, Wq, Wkv, Wproj, bproj, H=None, W=None, **_unused):
    in_maps = make_in_maps(x, Wq, Wkv, Wproj)
    res = run_spmd(in_maps)
    acc = np.zeros((N, C), np.float64)
    for r in res.results:
        acc += r["out"]
    out = acc.astype(np.float32) + np.asarray(bproj, np.float32)[None, :]
    return out.reshape(1, N, C)


if __name__ == "__main__":
    nc = build_nc()
    print("built ok")
